# revision 2
# baseline (speedup 1.0000x reference)
"""2-layer GCN (GCNConv x2) on 8 trn2 NeuronCores — dma_gather version.

Structure (SPMD-uniform across cores; per-core data in inputs):
  - nodes sharded 12500/core; phase A computes p1 = x @ W1 per shard
    (PE transpose of x tiles + matmul), writes fp16 table1 shard
    [NC_PAD=12544, 128]; AllGather table1.
  - Aggregation slots: chunks of <=32 dst nodes. Each chunk owns a fixed
    32-col psum window and 4 range-sections of 128 gather slots (one per
    owner-pair range, so gather indices fit dma_gather's int16) plus a
    32-slot self-loop section gathered from the LOCAL table shard.
  - Per psum group (16 chunks = 512 cols): 4 range dma_gathers (2048 idxs,
    256B rows) + 1 self dma_gather (512 idxs); 64 windowed matmuls
    (lhsT=block [128,128], rhs=M [128,32]) + 4 self matmuls with diagonal
    M blocks [128,128]; relu(ps+b1) -> h1; h2 = W2^T @ h1; transpose to
    row-major slots -> t2_local [S2, 128] f16 (cols 64: garbage pad).
  - AllGather table2; L2 same chunk structure (same M), gathers 128-wide
    padded h2 rows, matmuls use lhsT cols 0:64; psum [64, 512] -> out.
  - host: out[v] = outT[own(v)][:, slot_global(v)] + b2.
"""
import numpy as np

N_NODES = 100000
N_CORES = 8
NCN = N_NODES // N_CORES           # 12500
NC_PAD = 12544                     # 98*128 phase-A padded shard rows
IN_CH, HID, OUT_CH = 256, 128, 64
W = 32                             # dst nodes per chunk / psum window
GRP = 16                           # chunks per psum group (512 cols)
SEC = 128                          # gather slots per (chunk, range) section
NRNG = 4                           # owner-pair ranges (2 cores each)

_CACHE = {}


def _preprocess(edge_index):
    src = np.asarray(edge_index[0], dtype=np.int64)
    dst = np.asarray(edge_index[1], dtype=np.int64)
    deg = np.bincount(dst, minlength=N_NODES).astype(np.float64) + 1.0
    dinv = 1.0 / np.sqrt(deg)
    norm_e = (dinv[src] * dinv[dst]).astype(np.float32)   # non-self edges
    selfw = (dinv * dinv).astype(np.float32)              # self-loop weights

    own = src // NCN                                      # src owner core
    rng_e = (own // 2).astype(np.int64)                   # range 0..3
    row1 = (own % 2) * NC_PAD + (src - own * NCN)         # idx in L1 subtable

    # --- greedy chunking per core ---------------------------------------
    per_core = []
    for c in range(N_CORES):
        m = (dst >= c * NCN) & (dst < (c + 1) * NCN)
        dl = dst[m] - c * NCN
        cnt = np.zeros((NCN, NRNG), np.int64)
        np.add.at(cnt, (dl, rng_e[m]), 1)
        chunk_of = np.zeros(NCN, np.int64)
        pos_of = np.zeros(NCN, np.int64)
        ch, nd = 0, 0
        sums = np.zeros(NRNG, np.int64)
        for v in range(NCN):
            cv = cnt[v]
            if nd == W or np.any(sums + cv > SEC):
                ch += 1
                nd = 0
                sums[:] = 0
            chunk_of[v] = ch
            pos_of[v] = nd
            sums += cv
            nd += 1
        per_core.append((m, dl, chunk_of, pos_of, ch + 1))

    NCH = max(pc[4] for pc in per_core)
    NCH = ((NCH + GRP - 1) // GRP) * GRP
    NGRP = NCH // GRP
    S2 = NCH * W                                          # slot rows / owner
    assert 2 * S2 <= 32767 and 2 * NC_PAD <= 32767

    GCOLS = (NRNG * SEC * GRP + GRP * 2 * W) // 16        # idx cols per group (576)
    MCOLS = NRNG * GRP * W + GRP * 2 * W                  # m cols per group (3072)
    slot_global = np.zeros(N_NODES, np.int64)
    for c in range(N_CORES):
        _, _, chunk_of, pos_of, _ = per_core[c]
        slot_global[c * NCN : (c + 1) * NCN] = chunk_of * W + pos_of

    cores = []
    for c in range(N_CORES):
        m, dl, chunk_of, pos_of, _ = per_core[c]
        sl = src[m]
        nm = norm_e[m]
        r_e = rng_e[m]
        i1 = row1[m]
        ch_e = chunk_of[dl]
        pos_e = pos_of[dl]

        # order edges by (range, dst) -> sections contiguous
        order = np.lexsort((dl, r_e))
        sl, nm, r_e, i1, ch_e, pos_e = (
            sl[order], nm[order], r_e[order], i1[order], ch_e[order],
            pos_e[order])
        # position within (chunk, range) section
        key = r_e * NCH + ch_e
        uk, first = np.unique(key, return_index=True)
        sec_start = np.zeros(len(key), np.int64)
        sec_start[first] = first
        sec_start = np.maximum.accumulate(sec_start)
        sec_pos = np.arange(len(key)) - sec_start
        assert sec_pos.max() < SEC

        g_e = ch_e // GRP
        k_e = ch_e % GRP
        # idx tensor: [128, NGRP*GCOLS] int16 (16-wrapped, replicated x8)
        idx1 = np.zeros((16, NGRP * GCOLS), np.int16)
        idx2 = np.zeros((16, NGRP * GCOLS), np.int16)
        mval = np.zeros((128, NGRP * MCOLS), np.float16)
        # range sections; within group: call r occupies positions
        # [r*2048, (r+1)*2048); self call last (512 positions)
        base_g = g_e * GCOLS * 16
        pos_in_g = r_e * SEC * GRP + k_e * SEC + sec_pos   # 0..8191
        flat = base_g + pos_in_g
        idx1[flat % 16, flat // 16] = i1.astype(np.int16)
        own_s = sl // NCN
        i2 = (own_s % 2) * S2 + slot_global[sl]
        idx2[flat % 16, flat // 16] = i2.astype(np.int16)
        # M entries for range sections
        mcol = g_e * MCOLS + (r_e * GRP + k_e) * W + pos_e
        mval[sec_pos, mcol] = nm.astype(np.float16)

        # self sections: 64 slots/chunk (32 used + 32 pad) so matmul
        # partition bases stay in {0, 64}; chunk k pos p -> slot k*64+p
        v = np.arange(NCN)
        gs = chunk_of // GRP
        ks = chunk_of % GRP
        sp = ks * 2 * W + pos_of                           # 0..1023 within group
        flat_s = gs * GCOLS * 16 + NRNG * SEC * GRP + sp
        idx1s = v.astype(np.int16)                         # local row in t1_local
        idx2s = slot_global[c * NCN + v].astype(np.int16)  # local slot row
        idx1[flat_s % 16, flat_s // 16] = idx1s
        idx2[flat_s % 16, flat_s // 16] = idx2s
        # self M: diagonal blocks
        b_s = sp // 128
        p_s = sp % 128
        mcol_s = gs * MCOLS + NRNG * GRP * W + b_s * 128 + p_s
        mval[p_s, mcol_s] = selfw[c * NCN + v].astype(np.float16)

        cores.append(dict(
            idx1=np.ascontiguousarray(np.tile(idx1, (8, 1))),
            idx2=np.ascontiguousarray(np.tile(idx2, (8, 1))),
            m=np.ascontiguousarray(mval),
        ))
    return cores, NCH, NGRP, S2, slot_global


def _build_kernel(NCH, NGRP, S2, stage=4):
    import concourse.bass as bass
    import concourse.mybir as mybir
    from concourse import tile, library_config
    from concourse.masks import make_identity

    f16, f32, i16 = mybir.dt.float16, mybir.dt.float32, mybir.dt.int16
    GCOLS = (NRNG * SEC * GRP + GRP * 2 * W) // 16
    MCOLS = NRNG * GRP * W + GRP * 2 * W
    NBLK = NRNG * GRP + GRP * 2 * W // 128                # 72 blocks / group

    nc = bass.Bass(num_devices=N_CORES, num_swdge_queues=4)
    xt0_in = nc.dram_tensor("xt0", [128, NC_PAD], f32, kind="ExternalInput")
    xt1_in = nc.dram_tensor("xt1", [128, NC_PAD], f32, kind="ExternalInput")
    w1_in = nc.dram_tensor("w1", [IN_CH, HID], f32, kind="ExternalInput")
    w2_in = nc.dram_tensor("w2h", [HID, OUT_CH], f16, kind="ExternalInput")
    b1_in = nc.dram_tensor("b1col", [HID, 1], f32, kind="ExternalInput")
    idx1_in = nc.dram_tensor("idx1", [128, NGRP * GCOLS], i16, kind="ExternalInput")
    idx2_in = nc.dram_tensor("idx2", [128, NGRP * GCOLS], i16, kind="ExternalInput")
    m_in = nc.dram_tensor("m", [128, NGRP * MCOLS], f16, kind="ExternalInput")
    out_t = nc.dram_tensor("outT", [OUT_CH, S2], f32, kind="ExternalOutput")

    t1_local = nc.dram_tensor("t1_local", [NC_PAD, HID], f16, kind="Internal")
    table1 = nc.dram_tensor(
        "table1", [N_CORES * NC_PAD, HID], f16, kind="Internal", addr_space="Shared"
    )
    t2_local = nc.dram_tensor("t2_local", [S2, HID], f16, kind="Internal")
    table2 = nc.dram_tensor(
        "table2", [N_CORES * S2, HID], f16, kind="Internal", addr_space="Shared"
    )

    with tile.TileContext(nc) as tc:
        with (
            tc.tile_pool(name="const", bufs=1) as cpool,
            tc.tile_pool(name="xin", bufs=3) as xpool,
            tc.tile_pool(name="xt", bufs=3) as xtpool,
            tc.tile_pool(name="stage", bufs=4) as spool,
            tc.tile_pool(name="g", bufs=3) as gpool,
            tc.tile_pool(name="g2", bufs=3) as g2pool,
            tc.tile_pool(name="mi", bufs=3) as mpool,
            tc.tile_pool(name="h1", bufs=2) as hpool,
            tc.tile_pool(name="psum", bufs=4, space="PSUM") as pspool,
            tc.tile_pool(name="psumt", bufs=2, space="PSUM") as ptpool,
        ):
            nc.gpsimd.load_library(library_config.mlp)
            r2048 = nc.gpsimd.to_reg(SEC * GRP)                 # 2048
            r1024 = nc.gpsimd.to_reg(GRP * 2 * W)               # 1024

            ident = cpool.tile([128, 128], f32)
            make_identity(nc, ident[:])
            identh = cpool.tile([128, 128], f16, name="identh")
            make_identity(nc, identh[:])
            w1a = cpool.tile([128, HID], f32, name="w1a")
            w1b = cpool.tile([128, HID], f32, name="w1b")
            nc.sync.dma_start(out=w1a[:], in_=w1_in[0:128, :])
            nc.sync.dma_start(out=w1b[:], in_=w1_in[128:256, :])
            w2_sb = cpool.tile([HID, OUT_CH], f16, name="w2sb")
            nc.sync.dma_start(out=w2_sb[:], in_=w2_in[:])
            b1_sb = cpool.tile([HID, 1], f32, name="b1sb")
            nc.sync.dma_start(out=b1_sb[:], in_=b1_in[:])

            # ---------- phase A: p1 = x @ W1 (x pre-transposed on host) ----------
            for t in range(NC_PAD // 128):
                xT0 = xtpool.tile([128, 128], f32, tag="xt0")
                xT1 = xtpool.tile([128, 128], f32, tag="xt1")
                nc.sync.dma_start(out=xT0[:], in_=xt0_in[:, t * 128 : (t + 1) * 128])
                nc.sync.dma_start(out=xT1[:], in_=xt1_in[:, t * 128 : (t + 1) * 128])
                psp = pspool.tile([128, 512], f32, tag="ps")
                nc.tensor.matmul(out=psp[:, 0:HID], lhsT=xT0[:], rhs=w1a[:], start=True, stop=False)
                nc.tensor.matmul(out=psp[:, 0:HID], lhsT=xT1[:], rhs=w1b[:], start=False, stop=True)
                p1t = spool.tile([128, HID], f16, tag="p1")
                nc.scalar.copy(out=p1t[:], in_=psp[:, 0:HID])
                nc.sync.dma_start(out=t1_local[t * 128 : (t + 1) * 128, :], in_=p1t[:])

            if stage >= 1:
                nc.gpsimd.collective_compute(
                    "AllGather",
                    mybir.AluOpType.bypass,
                    replica_groups=[list(range(N_CORES))],
                    ins=[t1_local[:]],
                    outs=[table1[:]],
                )

            qn = [0]

            def gathers(gt, it, rows_per_owner, table, local_tab):
                for r in range(NRNG):
                    nc.gpsimd.dma_gather(
                        out_ap=gt[:, r * GRP : (r + 1) * GRP, :],
                        in_ap=table[2 * r * rows_per_owner : 2 * (r + 1) * rows_per_owner, :],
                        idxs_ap=it[:, r * SEC * GRP // 16 : (r + 1) * SEC * GRP // 16],
                        num_idxs=SEC * GRP,
                        num_idxs_reg=r2048,
                        elem_size=HID,
                        single_packet=False,
                        queue_num=qn[0] % 4,
                    )
                    qn[0] += 1
                nc.gpsimd.dma_gather(
                    out_ap=gt[:, NRNG * GRP : NBLK, :],
                    in_ap=local_tab[:],
                    idxs_ap=it[:, NRNG * SEC * GRP // 16 : GCOLS],
                    num_idxs=GRP * 2 * W,
                    num_idxs_reg=r1024,
                    elem_size=HID,
                    single_packet=False,
                    queue_num=qn[0] % 4,
                )
                qn[0] += 1

            def agg_matmuls(ps, gt, m_t, prow, fcols):
                # each psum window = ONE consecutive 5-matmul accumulation
                # chain (PE loses the accumulation context if any other
                # matmul group runs in between)
                for k in range(GRP):
                    for r in range(NRNG):
                        nc.tensor.matmul(
                            out=ps[:prow, k * W : (k + 1) * W],
                            lhsT=gt[:, r * GRP + k, fcols[0] : fcols[1]],
                            rhs=m_t[:, (r * GRP + k) * W : (r * GRP + k + 1) * W],
                            start=(r == 0),
                            stop=False,
                        )
                    b, p0 = (k * 2 * W) // 128, (k * 2 * W) % 128
                    sbase = NRNG * GRP * W + b * 128
                    nc.tensor.matmul(
                        out=ps[:prow, k * W : (k + 1) * W],
                        lhsT=gt[p0 : p0 + W, NRNG * GRP + b, fcols[0] : fcols[1]],
                        rhs=m_t[p0 : p0 + W, sbase + p0 : sbase + p0 + W],
                        start=False,
                        stop=True,
                    )

            # ---------- L1 aggregation ----------
            for g in range(NGRP if stage >= 2 else 0):
                it = mpool.tile([128, GCOLS], i16, tag="i1")
                nc.sync.dma_start(out=it[:], in_=idx1_in[:, g * GCOLS : (g + 1) * GCOLS])
                m_t = mpool.tile([128, MCOLS], f16, tag="m1")
                nc.sync.dma_start(out=m_t[:], in_=m_in[:, g * MCOLS : (g + 1) * MCOLS])
                gt = gpool.tile([128, NBLK, HID], f16, tag="g")
                gathers(gt, it, NC_PAD, table1, t1_local)
                ps = pspool.tile([128, 512], f32, tag="ps")
                agg_matmuls(ps, gt, m_t, 128, (0, HID))
                h1 = hpool.tile([128, 512], f16, tag="h1")
                nc.scalar.activation(
                    out=h1[:], in_=ps[:],
                    func=mybir.ActivationFunctionType.Relu,
                    bias=b1_sb[:, :1], scale=1.0,
                )
                ps2 = pspool.tile([128, 512], f32, tag="ps")
                nc.tensor.matmul(out=ps2[:OUT_CH, :], lhsT=w2_sb[:], rhs=h1[:], start=True, stop=True)
                g2s = spool.tile([OUT_CH, 512], f16, tag="g2s")
                nc.scalar.copy(out=g2s[:], in_=ps2[:OUT_CH, :])
                for q in range(4):
                    ps3 = ptpool.tile([128, 512], f16, tag="psh")
                    nc.tensor.transpose(
                        out=ps3[:, q * 128 : q * 128 + OUT_CH],
                        in_=g2s[:, q * 128 : (q + 1) * 128],
                        identity=identh[:OUT_CH, :OUT_CH],
                    )
                    t2t = spool.tile([128, OUT_CH], f16, tag="t2t")
                    nc.scalar.copy(out=t2t[:], in_=ps3[:, q * 128 : q * 128 + OUT_CH])
                    r0 = g * 512 + q * 128
                    nc.sync.dma_start(out=t2_local[r0 : r0 + 128, 0:OUT_CH], in_=t2t[:])

            if stage >= 3:
                nc.gpsimd.collective_compute(
                    "AllGather",
                    mybir.AluOpType.bypass,
                    replica_groups=[list(range(N_CORES))],
                    ins=[t2_local[:]],
                    outs=[table2[:]],
                )

            # ---------- L2 aggregation ----------
            for g in range(NGRP if stage >= 4 else 0):
                it = mpool.tile([128, GCOLS], i16, tag="i2")
                nc.sync.dma_start(out=it[:], in_=idx2_in[:, g * GCOLS : (g + 1) * GCOLS])
                m_t = mpool.tile([128, MCOLS], f16, tag="m2")
                nc.sync.dma_start(out=m_t[:], in_=m_in[:, g * MCOLS : (g + 1) * MCOLS])
                gt2 = g2pool.tile([128, NBLK, HID], f16, tag="g2")
                gathers(gt2, it, S2, table2, t2_local)
                ps = pspool.tile([128, 512], f32, tag="ps")
                agg_matmuls(ps, gt2, m_t, OUT_CH, (0, OUT_CH))
                osb = spool.tile([OUT_CH, 512], f32, tag="osb")
                nc.scalar.copy(out=osb[:], in_=ps[:OUT_CH, :])
                nc.sync.dma_start(out=out_t[:, g * 512 : (g + 1) * 512], in_=osb[:])

    from tile_patch_embedded import split_multi_waits

    split_multi_waits(nc)
    from concourse.library_overlay import lower_extended_insts

    lower_extended_insts(nc)
    return nc


# --- embedded copy of the walrus multi-wait workaround (self-contained) ---
import sys as _sys
import types as _types

_tp_src = '''
import concourse.mybir as mybir

def split_multi_waits(nc, max_waits=1):
    n_split = 0
    for fn in nc.m.functions:
        for blk in fn.blocks:
            insts = blk.instructions
            i = 0
            while i < len(insts):
                inst = insts[i]
                si = inst.sync_info
                waits = list(si.on_wait) if si is not None else []
                if len(waits) > max_waits:
                    keep = waits[:max_waits]
                    extra = waits[max_waits:]
                    si.on_wait = keep
                    new_nops = []
                    for k in range(0, len(extra), max_waits):
                        nop = mybir.InstNoOp(
                            name=f"{inst.name}-xw{k}",
                            sync_info=mybir.SyncInfo(
                                on_wait=extra[k : k + max_waits], on_update=[]
                            ),
                            bass_nofuse=True,
                            engine=inst.engine,
                        )
                        new_nops.append(nop)
                        nc.register_instruction(nop, overwrite=True)
                    insts[i:i] = new_nops
                    i += len(new_nops)
                    n_split += 1
                i += 1
    return n_split
'''
_tp_mod = _types.ModuleType("tile_patch_embedded")
exec(_tp_src, _tp_mod.__dict__)
_sys.modules["tile_patch_embedded"] = _tp_mod


def _prep_all(x, edge_index, W1, b1, W2, b2, stage=4):
    """Build (nc, in_maps, slot_global) for the given inputs."""
    x = np.asarray(x, dtype=np.float32)
    W1 = np.asarray(W1, dtype=np.float32)
    W2 = np.asarray(W2, dtype=np.float32)
    b1 = np.asarray(b1, dtype=np.float32)

    ekey = (hash(np.asarray(edge_index)[:, ::997].tobytes()), stage)
    if ekey in _CACHE:
        cores, NCH, NGRP, S2, slot_global, nc = _CACHE[ekey]
    else:
        pkey = ekey[0]
        if ("prep", pkey) in _CACHE:
            cores, NCH, NGRP, S2, slot_global = _CACHE[("prep", pkey)]
        else:
            cores, NCH, NGRP, S2, slot_global = _preprocess(edge_index)
            _CACHE[("prep", pkey)] = (cores, NCH, NGRP, S2, slot_global)
        nc = _build_kernel(NCH, NGRP, S2, stage=stage)
        _CACHE[ekey] = (cores, NCH, NGRP, S2, slot_global, nc)

    b1col = np.ascontiguousarray(b1.reshape(HID, 1))
    w2h = W2.astype(np.float16)
    in_maps = []
    for c in range(N_CORES):
        xs = np.zeros((NC_PAD, IN_CH), np.float32)
        xs[:NCN] = x[c * NCN : (c + 1) * NCN]
        xT = np.ascontiguousarray(xs.T)
        in_maps.append(
            dict(
                xt0=xT[0:128], xt1=xT[128:256], w1=W1, w2h=w2h, b1col=b1col,
                idx1=cores[c]["idx1"], idx2=cores[c]["idx2"], m=cores[c]["m"],
            )
        )
    return nc, in_maps, slot_global


def build_for_timing(inputs, stage=4):
    nc, in_maps, _ = _prep_all(
        inputs["x"], inputs["edge_index"], inputs["W1"], inputs["b1"],
        inputs["W2"], inputs["b2"], stage=stage,
    )
    return nc, in_maps


def kernel(x, edge_index, W1, b1, W2, b2):
    from concourse.bass_utils import run_bass_kernel_spmd

    b2 = np.asarray(b2, dtype=np.float32)
    nc, in_maps, slot_global = _prep_all(x, edge_index, W1, b1, W2, b2)
    res = run_bass_kernel_spmd(nc, in_maps, core_ids=list(range(N_CORES)))
    outs = np.stack([res.results[c]["outT"] for c in range(N_CORES)])  # [8, 64, S2]
    own = np.arange(N_NODES) // NCN
    out = outs[own, :, slot_global].astype(np.float32)  # [N, 64]
    out = out + b2[None, :]
    return out

